# revision 19
# baseline (speedup 1.0000x reference)
"""Trainium2 Bass kernel for nn_CQDBeam (DistMult scoring + segment-sum + top-k).

Math refactor: term_tgt_scores[b, n] = H[b] . emb[n] + c[b], where
  H[b] = sum_{e: term(e)=b} sign(e) * (emb[head_e] * rel[rel_e])   [128, 64]
  c[b] = sum_{e: term(e)=b} src_score[e]                            (per-term const)
Since c[b] is constant per term, top-k selection is invariant to it; the device
scores H @ embT only. Entities are sharded 8 ways (25088 per core, padded).

Device per core:
  - gather hq rows (indirect DMA), build one-hot S via iota compare,
    matmul-accumulate -> [H^T; c^T] in PSUM  (the segment-sum over edges)
  - stream 49 fp32 matmul tiles [128 terms, 512 entities] and reduce each to a
    16-ary segmented max -> L1 [128, 1568]
  - per-lane bisection on L1 for the largest threshold T with
    count(L1 >= T) >= 128  (fused count via tensor_scalar accum_out)
  - output mask = (L1 >= T - eps) [128, 1568] u8
Invariant: count_L1(>=T) >= 128 implies >= 128 entities score >= T, so
T <= (128th best entity score); hence every top-128 entity's 16-group has
L1 >= T and is in the mask. Exact containment, unconditionally.

Host: decode masks -> candidate entity groups (~140 groups x 16 per term per
core), recompute exact fp32 values from the device-computed H/c, merge the 8
cores, emit top-128 values + indices per term (jax tie-break: lower index).
"""

import numpy as np

import concourse.bass as bass
import concourse.mybir as mybir
import concourse.tile as tile
from concourse.tile import TileContext
from concourse.vector_clock import ScopedClock
from concourse.bass_utils import run_bass_kernel_spmd

# ---------------------------------------------------------------------------
# Walrus in this container accepts only ONE sync wait per instruction.
# TileContext attaches multiple waits; split the excess onto preceding
# same-engine EventSemaphore waits (in-order execution makes this equivalent).
def _patched_drain_and_barrier(self, tick_clock, wait_clock):
    drain0 = self.nc.sync.drain().ins
    wait_clock.add_sem_waits(drain0, ScopedClock({None: tick_clock.global_clock}))
    self.nc.all_engine_barrier()
    popped = self.nc._tile_sem_poison_stack.pop()
    assert popped is self._sem_poison
    self.nc.clear_and_free_semaphores(list(self.sems.allocated().values()))
    self.nc.all_engine_barrier()


TileContext._drain_and_barrier = _patched_drain_and_barrier


def split_waits(nc):
    for f in nc.m.functions:
        for bb in f.blocks:
            insts = list(bb.instructions)
            out = []
            for inst in insts:
                si = inst.sync_info
                waits = list(si.on_wait) if si and si.on_wait else []
                if len(waits) > 1:
                    for k, w in enumerate(waits[:-1]):
                        ev = mybir.InstEventSemaphore(
                            name=f"wsplit-{inst.name}-{k}", ins=[], outs=[])
                        ev.engine = inst.engine
                        ev.sync_info = mybir.SyncInfo(on_wait=[w], on_update=[])
                        out.append(ev)
                    si.on_wait = [waits[-1]]
                out.append(inst)
            bb.instructions = out


# ---------------------------------------------------------------------------
N_ENT = 200_000
N_REL = 500
DIM = 64
B = 128
E = 1024
BEAM = 128
N_CORES = 8
N_LOC = 25088            # per-core entity slice (49 * 512), 25088*8 = 200704
N_TILE = 512
N_TILES = N_LOC // N_TILE   # 49
L1_ARY = 16
L1_W = N_LOC // L1_ARY      # 1568
BISECT = 12

f32 = mybir.dt.float32
i32 = mybir.dt.int32
u8 = mybir.dt.uint8


def build_nc():
    nc = bass.Bass("TRN2")
    embT = nc.dram_tensor("embT", [DIM, N_LOC], mybir.dt.float32r, kind="ExternalInput")
    hq_in = nc.dram_tensor("hq", [E // 128, 128, DIM], f32, kind="ExternalInput")
    e2t = nc.dram_tensor("e2t", [E], i32, kind="ExternalInput")
    neg = nc.dram_tensor("neg", [E], i32, kind="ExternalInput")
    src = nc.dram_tensor("src", [E], f32, kind="ExternalInput")
    maskout = nc.dram_tensor("maskout", [B, L1_W], u8, kind="ExternalOutput")
    hext = nc.dram_tensor("hext", [DIM + 1, B], f32, kind="ExternalOutput")

    with TileContext(nc) as tc:
        with (
            tc.tile_pool(name="setup", bufs=1) as sp,
            tc.tile_pool(name="emb", bufs=4) as ep,
            tc.tile_pool(name="psum", bufs=1, space="PSUM") as pp,
            tc.tile_pool(name="work", bufs=1) as wp,
        ):
            # ---------------- setup: H^T|c ----------------
            EC = E // 128  # 8 edge chunks
            e2t_sb = sp.tile([128, EC], i32)
            neg_sb = sp.tile([128, EC], i32)
            src_sb = sp.tile([128, EC], f32)
            for t_sb, t_dr in ((e2t_sb, e2t), (neg_sb, neg)):
                nc.sync.dma_start(out=t_sb[:], in_=t_dr[:].rearrange("(c p) -> p c", p=128))
            nc.sync.dma_start(out=src_sb[:], in_=src[:].rearrange("(c p) -> p c", p=128))

            hq_sb = sp.tile([128, EC, DIM], f32)
            nc.sync.dma_start(out=hq_sb[:], in_=hq_in[:].rearrange("c p d -> p c d"))

            e2t_f = sp.tile([128, EC], f32)
            sign_f = sp.tile([128, EC], f32)
            signsrc = sp.tile([128, EC], f32)
            nc.vector.tensor_copy(e2t_f[:], e2t_sb[:])
            # sign = 1 - 2*neg
            nc.vector.tensor_scalar(sign_f[:], neg_sb[:], -2.0, 1.0,
                                    op0=mybir.AluOpType.mult, op1=mybir.AluOpType.add)
            nc.vector.tensor_tensor(out=signsrc[:], in0=sign_f[:], in1=src_sb[:],
                                    op=mybir.AluOpType.mult)

            # rhs_ext[:, c, 0:64] = hq ; [:, c, 64] = sign*src
            rhs_ext = sp.tile([128, EC, DIM + 1], f32)
            nc.vector.tensor_copy(rhs_ext[:, :, 0:DIM], hq_sb[:])
            nc.vector.tensor_copy(rhs_ext[:, :, DIM], signsrc[:])

            iota128 = sp.tile([128, 128], i32)
            nc.gpsimd.iota(iota128[:], pattern=[[1, 128]], base=0, channel_multiplier=0)
            iota128f = sp.tile([128, 128], f32)
            nc.vector.tensor_copy(iota128f[:], iota128[:])

            psum_ht = pp.tile([DIM + 1, 128], f32, space="PSUM", tag="ps", bufs=2)
            s_chunk = [sp.tile([128, 128], f32, tag=f"s{c}", name=f"s{c}")
                       for c in range(EC)]
            for c in range(EC):
                # S_c[e, b] = (iota_b == e2t_e) * sign_e
                nc.vector.tensor_scalar(
                    s_chunk[c][:], iota128f[:], e2t_f[:, c:c + 1], sign_f[:, c:c + 1],
                    op0=mybir.AluOpType.is_equal, op1=mybir.AluOpType.mult)
            for c in range(EC):
                nc.tensor.matmul(out=psum_ht[:], lhsT=rhs_ext[:, c, :], rhs=s_chunk[c][:],
                                 start=(c == 0), stop=(c == EC - 1))
            ht_sb = sp.tile([DIM + 1, 128], f32)
            nc.vector.tensor_copy(ht_sb[:], psum_ht[:])
            nc.sync.dma_start(out=hext[:], in_=ht_sb[:])

            # ---------------- stream: scores -> L1 (16-ary max) ----------------
            # 4-tile groups: one DMA + one grouped PSUM reduce per 2048 entities
            f32r = mybir.dt.float32r
            GRP = 4
            GW = GRP * N_TILE          # 2048
            N_GRP = N_LOC // GW        # 12 groups + 1 tail tile
            L1 = wp.tile([128, L1_W], f32)
            ht_r = sp.tile([DIM, 128], f32r)
            nc.scalar.copy(ht_r[:], ht_sb[0:DIM, :])
            for g in range(N_GRP + 1):
                gw = GW if g < N_GRP else N_LOC - N_GRP * GW
                if gw <= 0:
                    break
                et = ep.tile([DIM, GW], f32r, tag="embtile")
                nc.sync.dma_start(out=et[:, :gw], in_=embT[:, g * GW:g * GW + gw])
                ps = pp.tile([128, GW], f32, space="PSUM", tag="ps", bufs=2)
                for k in range(gw // N_TILE):
                    nc.tensor.matmul(
                        out=ps[:, k * N_TILE:(k + 1) * N_TILE], lhsT=ht_r[:],
                        rhs=et[:, k * N_TILE:(k + 1) * N_TILE],
                        start=True, stop=True)
                nc.vector.tensor_reduce(
                    out=L1[:, g * (GW // L1_ARY):g * (GW // L1_ARY) + gw // L1_ARY],
                    in_=ps[:, :gw].rearrange("p (c l) -> p c l", l=L1_ARY),
                    axis=mybir.AxisListType.X, op=mybir.AluOpType.max)

            # ---------------- bisect threshold on L2 = 4-max(L1) ----------------
            L2_W = L1_W // 4
            L2 = wp.tile([128, L2_W], f32)
            nc.vector.tensor_reduce(out=L2[:], in_=L1[:].rearrange("p (c l) -> p c l", l=4),
                                    axis=mybir.AxisListType.X, op=mybir.AluOpType.max)
            lo = wp.tile([128, 1], f32)
            hi = wp.tile([128, 1], f32)
            mid = wp.tile([128, 1], f32)
            cnt = wp.tile([128, 1], f32)
            ge = wp.tile([128, 1], u8)
            tmp = wp.tile([128, L2_W], f32)
            nc.vector.tensor_reduce(out=lo[:], in_=L2[:], axis=mybir.AxisListType.X,
                                    op=mybir.AluOpType.min)
            nc.vector.tensor_reduce(out=hi[:], in_=L2[:], axis=mybir.AxisListType.X,
                                    op=mybir.AluOpType.max)
            nc.vector.tensor_scalar(mid[:], lo[:], hi[:], 0.5,
                                    op0=mybir.AluOpType.add, op1=mybir.AluOpType.mult)
            for _ in range(BISECT):
                nc.vector.tensor_scalar(tmp[:], L2[:], mid[:], 0.0,
                                        op0=mybir.AluOpType.is_ge,
                                        op1=mybir.AluOpType.add, accum_out=cnt[:])
                nc.vector.tensor_scalar(ge[:], cnt[:], float(BEAM), None,
                                        op0=mybir.AluOpType.is_ge)
                nc.vector.select(lo[:], ge[:], mid[:], lo[:])
                nc.vector.select(hi[:], ge[:], hi[:], mid[:])
                nc.vector.tensor_scalar(mid[:], lo[:], hi[:], 0.5,
                                        op0=mybir.AluOpType.add, op1=mybir.AluOpType.mult)

            # T' = lo - 1e-5*max|L1| - 1e-30  (margin vs fp association)
            amax = wp.tile([128, 1], f32)
            nc.vector.tensor_reduce(out=amax[:], in_=L2[:], axis=mybir.AxisListType.X,
                                    op=mybir.AluOpType.max, apply_absolute_value=True)
            marg = wp.tile([128, 1], f32)
            nc.vector.tensor_scalar(marg[:], amax[:], -0.015625, -1e-30,
                                    op0=mybir.AluOpType.mult, op1=mybir.AluOpType.add)
            nc.vector.tensor_tensor(out=marg[:], in0=lo[:], in1=marg[:],
                                    op=mybir.AluOpType.add)

            maskt = wp.tile([128, L1_W], u8)
            nc.vector.tensor_scalar(maskt[:], L1[:], marg[:], None,
                                    op0=mybir.AluOpType.is_ge)
            nc.sync.dma_start(out=maskout[:], in_=maskt[:])
    split_waits(nc)
    return nc


_NC_CACHE = {}
LAST_RESULT = None


def _get_nc():
    if "nc" not in _NC_CACHE:
        _NC_CACHE["nc"] = build_nc()
    return _NC_CACHE["nc"]


def kernel(head_id, rel_id, negation, edge_to_term, src_score, entity_emb, rel_emb):
    head_id = np.asarray(head_id, dtype=np.int32)
    rel_id = np.asarray(rel_id, dtype=np.int32)
    negation = np.asarray(negation, dtype=np.int32)
    edge_to_term = np.asarray(edge_to_term, dtype=np.int32)
    src_score = np.ascontiguousarray(np.asarray(src_score, dtype=np.float32))
    entity_emb = np.ascontiguousarray(np.asarray(entity_emb, dtype=np.float32))
    rel_emb = np.ascontiguousarray(np.asarray(rel_emb, dtype=np.float32))

    # shard: embT padded to [64, 200704], per-core contiguous [64, 25088]
    embT = np.zeros((DIM, N_CORES * N_LOC), dtype=np.float32)
    embT[:, :N_ENT] = entity_emb.T
    # pre-round to fp32r (tf32: keep 10 mantissa bits, round-to-nearest-even)
    u = embT.view(np.uint32)
    u += 0x1000 + ((u >> 13) & 1)
    u &= np.uint32(0xFFFFE000)
    # hq = emb[head]*rel[relid], laid out [chunk, partition, dim] (edge = c*128+p)
    hq = (entity_emb[head_id] * rel_emb[rel_id]).reshape(E // 128, 128, DIM)
    hq = np.ascontiguousarray(hq)
    shared = dict(
        hq=hq, e2t=edge_to_term, neg=negation, src=src_score)
    in_maps = [
        dict(embT=np.ascontiguousarray(embT[:, i * N_LOC:(i + 1) * N_LOC]), **shared)
        for i in range(N_CORES)
    ]
    nc = _get_nc()
    res = run_bass_kernel_spmd(nc, in_maps, core_ids=list(range(N_CORES)))
    global LAST_RESULT
    LAST_RESULT = res

    # ---------------- host merge ----------------
    hx = res.results[0]["hext"]          # [65, 128] (identical on all cores)
    H = np.ascontiguousarray(hx[:DIM, :].T)  # [128, 64]
    c = hx[DIM, :].copy()                # [128]

    out_val = np.empty((B, BEAM), dtype=np.float32)
    out_idx = np.empty((B, BEAM), dtype=np.int32)
    degenerate = np.abs(H).sum(axis=1) == 0.0

    # gather candidates (lane, global entity id, value) across cores
    lane_vals = [[] for _ in range(B)]
    lane_ids = [[] for _ in range(B)]
    r16 = np.arange(L1_ARY, dtype=np.int64)
    for i in range(N_CORES):
        m = res.results[i]["maskout"] != 0   # [128, 1568]
        base = i * N_LOC
        rows, gids = np.nonzero(m)
        if len(rows) == 0:
            continue
        pos = gids[:, None].astype(np.int64) * L1_ARY + r16[None, :] + base  # [K, 16]
        valid = pos < N_ENT
        gath = entity_emb[np.minimum(pos, N_ENT - 1)]      # [K, 16, 64]
        vals = np.einsum("ktd,kd->kt", gath, H[rows], optimize=True)  # [K, 16]
        for b in range(B):
            sel = rows == b
            if not sel.any():
                continue
            v = vals[sel][valid[sel]]
            p = pos[sel][valid[sel]]
            lane_vals[b].append(v)
            lane_ids[b].append(p)

    for b in range(B):
        if degenerate[b]:
            out_val[b] = c[b]
            out_idx[b] = np.arange(BEAM, dtype=np.int32)
            continue
        if lane_vals[b]:
            v = np.concatenate(lane_vals[b]) + np.float32(c[b])
            p = np.concatenate(lane_ids[b])
        else:
            v = np.empty(0, np.float32)
            p = np.empty(0, np.int64)
        if len(v) < BEAM:
            # safety net (should not happen): brute-force this term
            v = entity_emb @ H[b] + np.float32(c[b])
            p = np.arange(N_ENT, dtype=np.int64)
        order = np.lexsort((p, -v))[:BEAM]
        out_val[b] = v[order]
        out_idx[b] = p[order].astype(np.int32)

    return out_val, out_idx
